# revision 1
# baseline (speedup 1.0000x reference)
"""Paged-attention decode (GQA + ALiBi) Bass kernel for 8 Trainium2 cores.

Problem shape (hardcoded):
  query        [64, 32, 128] f32
  key_cache    [8192, 8, 16, 128] f32
  value_cache  [8192, 8, 16, 128] f32
  block_tables [64, 128] i32
  seq_lens     [64] i32
  out          [64, 32, 128] f32

Sharding: data-parallel over sequences. 64 seqs -> 8 cores x 8 slots; seqs are
sorted by length and dealt snake-wise so every core's slot j has a similar
chunk count. One SPMD program is value-specialized only on the per-slot chunk
counts CNT[0..7] (max over cores); everything else (block ids, ALiBi
rel/mask rows, q) flows in as per-core input data, so a single NEFF runs on
all 8 cores.

Per (slot, chunk of 128 positions = 8 KV blocks):
  - indirect-DMA gather of 8 K blocks and 8 V blocks (64KB contiguous each)
    into SBUF laid out [ (block,l) partitions, (kvh, d) free ]
  - per kv head: PE transpose K -> K^T [d, l], then matmul
    scoresT[l, g] += K^T(stationary) @ qT(moving, N=4)
  - one bias matmul adds ALiBi slope*rel + mask via a rank-2 update
  - ACT exp -> probs [l=128, 32 heads]
  - per kv head: outT[d, g] += V(natural, stationary) @ probs(moving, N=4)
    accumulated in PSUM across chunks; denominator via ones-vector matmul
Epilogue per slot: PE transpose outT -> [h, d], multiply by 1/denom, DMA out.

Softmax uses no max-subtraction: logits = q.k*scale + alibi <= ~10 here
(alibi <= 0, q.k*scale ~ N(0,1)), so exp never overflows; masked positions
get -1e30 -> exp == 0 exactly.
"""

import os
import numpy as np

S, H, KVH, G, D = 64, 32, 8, 4, 128
BS, L, NBLOCKS = 16, 2048, 8192
N_CORES = 8
CH = 128            # positions per chunk
BPC = CH // BS      # blocks per chunk
NEG = -1.0e30

_prog_cache = {}
LAST_NC = None      # for test harnesses: the last built Bass module


def _build_program(cnt):
    """Build the SPMD Bass program for per-slot chunk counts `cnt` (len 8)."""
    from contextlib import ExitStack

    import concourse.bass as bass
    import concourse.tile as tile
    from concourse import bacc, mybir
    from concourse.masks import make_identity

    f32 = mybir.dt.float32
    i32 = mybir.dt.int32
    tot = sum(cnt)
    cum = [0]
    for c in cnt:
        cum.append(cum[-1] + c)

    nc = bacc.Bacc(
        "TRN2",
        target_bir_lowering=False,
        debug=False,
        enable_asserts=False,
        num_devices=N_CORES,
    )
    kc_d = nc.dram_tensor("kc", [NBLOCKS, KVH, BS, D], f32, kind="ExternalInput")
    vc_d = nc.dram_tensor("vc", [NBLOCKS, KVH, BS, D], f32, kind="ExternalInput")
    qT_d = nc.dram_tensor("qT", [D, 8 * H], f32, kind="ExternalInput")
    ko_d = nc.dram_tensor("ko", [128, tot], i32, kind="ExternalInput")
    rm_d = nc.dram_tensor("rm", [2, tot * CH], f32, kind="ExternalInput")
    so_d = nc.dram_tensor("so", [2, H], f32, kind="ExternalInput")
    out_d = nc.dram_tensor("out", [8, H, D], f32, kind="ExternalOutput")

    with ExitStack() as ctx:
        tc = ctx.enter_context(tile.TileContext(nc))
        const = ctx.enter_context(tc.tile_pool(name="const", bufs=1))
        kvp = ctx.enter_context(tc.tile_pool(name="kv", bufs=4))
        ktp = ctx.enter_context(tc.tile_pool(name="kt", bufs=3))
        prp = ctx.enter_context(tc.tile_pool(name="pr", bufs=3))
        epp = ctx.enter_context(tc.tile_pool(name="ep", bufs=2))
        psT = ctx.enter_context(tc.tile_pool(name="psT", bufs=3, space="PSUM"))
        psS = ctx.enter_context(tc.tile_pool(name="psS", bufs=2, space="PSUM"))
        psA = ctx.enter_context(tc.tile_pool(name="psA", bufs=2, space="PSUM"))
        psD = ctx.enter_context(tc.tile_pool(name="psD", bufs=1, space="PSUM"))

        ident = const.tile([128, 128], f32)
        make_identity(nc, ident[:])
        ones = const.tile([128, 1], f32)
        nc.gpsimd.memset(ones[:], 1.0)
        qT_s = const.tile([D, 8 * H], f32)
        nc.sync.dma_start(qT_s[:], qT_d.ap())
        ko_s = const.tile([128, tot], i32)
        nc.sync.dma_start(ko_s[:], ko_d.ap())
        rm_s = const.tile([2, tot * CH], f32)
        nc.sync.dma_start(rm_s[:], rm_d.ap())
        so_s = const.tile([2, H], f32)
        nc.sync.dma_start(so_s[:], so_d.ap())

        for j in range(8):
            acc = psA.tile([128, H], f32)  # outT[d, h] accumulator
            den = psD.tile([H, 1], f32)  # softmax denominator per head
            for t in range(cnt[j]):
                ct = cum[j] + t
                ksb = kvp.tile([128, KVH * D], f32, tag="k")
                vsb = kvp.tile([128, KVH * D], f32, tag="v")
                for csb, cd in ((ksb, kc_d), (vsb, vc_d)):
                    # canonical indirect1d gather: one index per partition
                    # row, each fetching one contiguous [D] row. in_ viewed
                    # [NBLOCKS*KVH*BS, D] => coef = D; host index encodes
                    # block*BS + l; element_offset picks the kv head.
                    for h in range(KVH):
                        nc.gpsimd.indirect_dma_start(
                            out=csb[:, h * D : (h + 1) * D],
                            out_offset=None,
                            in_=cd.ap().rearrange("b h l d -> (b h l) d"),
                            in_offset=bass.IndirectOffsetOnAxis(
                                ap=ko_s[:, ct : ct + 1], axis=0
                            ),
                            element_offset=h * BS * D,
                        )
                kt = ktp.tile([128, KVH * D], f32)
                for h in range(KVH):
                    tp = psT.tile([128, 128], f32)
                    nc.tensor.transpose(
                        tp[:], ksb[:, h * D : (h + 1) * D], ident[:]
                    )
                    nc.vector.tensor_copy(kt[:, h * D : (h + 1) * D], tp[:])
                # bias first: one start=True writer for the whole PSUM zero
                # region (start marks the full 2KB region pending-zero, so it
                # must be unique and first); QK matmuls then accumulate.
                sc = psS.tile([128, H], f32)
                nc.tensor.matmul(
                    sc[:],
                    lhsT=rm_s[:, ct * CH : (ct + 1) * CH],
                    rhs=so_s[:],
                    start=True,
                    stop=False,
                )
                for h in range(KVH):
                    nc.tensor.matmul(
                        sc[:, G * h : G * (h + 1)],
                        lhsT=kt[:, h * D : (h + 1) * D],
                        rhs=qT_s[:, j * H + G * h : j * H + G * (h + 1)],
                        start=False,
                        stop=h == KVH - 1,
                    )
                pr = prp.tile([128, H], f32)
                nc.scalar.activation(
                    pr[:], sc[:], mybir.ActivationFunctionType.Exp
                )
                first, last = t == 0, t == cnt[j] - 1
                for h in range(KVH):
                    nc.tensor.matmul(
                        acc[:, G * h : G * (h + 1)],
                        lhsT=vsb[:, h * D : (h + 1) * D],
                        rhs=pr[:, G * h : G * (h + 1)],
                        start=first and h == 0,
                        stop=last and h == KVH - 1,
                    )
                nc.tensor.matmul(
                    den[:],
                    lhsT=pr[:],
                    rhs=ones[:],
                    start=first,
                    stop=last,
                )
            # epilogue: outT [d, h] -> [h, d], divide by denom, store
            oT = epp.tile([128, H], f32, tag="oT")
            nc.vector.tensor_copy(oT[:], acc[:])
            rec = epp.tile([H, 1], f32, tag="rec")
            nc.vector.reciprocal(rec[:], den[:])
            of = psT.tile([H, 128], f32, tag="tp")
            nc.tensor.transpose(of[:], oT[:], ident[:])
            osb = epp.tile([H, 128], f32, tag="osb")
            nc.vector.tensor_scalar_mul(osb[:], of[:], rec[:])
            nc.sync.dma_start(out_d.ap()[j], osb[:])

    nc.compile()
    return nc


def _prep(
    query,
    key_cache,
    value_cache,
    scale,
    block_tables,
    seq_lens,
    alibi_slopes,
):
    q = np.asarray(query, dtype=np.float32)
    kc = np.ascontiguousarray(np.asarray(key_cache, dtype=np.float32))
    vc = np.ascontiguousarray(np.asarray(value_cache, dtype=np.float32))
    bt = np.asarray(block_tables, dtype=np.int32)
    sl = np.asarray(seq_lens, dtype=np.int64)
    slope = np.asarray(alibi_slopes, dtype=np.float32)
    sc_f = float(np.asarray(scale))

    nch = np.maximum(1, -(-sl // CH))  # ceil, >= 1
    order = np.argsort(-nch, kind="stable")
    assign = np.empty((8, N_CORES), np.int64)  # [slot, core] -> seq idx
    for j in range(8):
        grp = order[j * 8 : (j + 1) * 8]
        assign[j] = grp if j % 2 == 0 else grp[::-1]
    cnt = tuple(int(nch[assign[j]].max()) for j in range(8))
    tot = sum(cnt)
    cum = np.concatenate([[0], np.cumsum(cnt)])

    # per-core input tensors
    so = np.stack([slope, np.ones(H, np.float32)]).astype(np.float32)  # [2, 32]
    in_maps = []
    for c in range(N_CORES):
        qT = np.zeros((D, 8 * H), np.float32)
        ko = np.zeros((128, tot), np.int32)
        rm = np.zeros((2, tot * CH), np.float32)
        for j in range(8):
            s = int(assign[j, c])
            qT[:, j * H : (j + 1) * H] = (q[s] * sc_f).T  # [128, 32]
            n = int(cnt[j])
            # per-partition-row gather indices: partition p of chunk t maps
            # to block bt[s, t*BPC + p//BS], row l = p % BS; the device view
            # is [NBLOCKS*KVH*BS, D] rows, head offset added on device.
            # (padded chunks use whatever the block table holds -- valid
            # ids, contributions masked to zero)
            blk = bt[s, : n * BPC].reshape(n, BPC).astype(np.int64)  # [n, 8]
            rows = np.repeat(blk * KVH * BS, BS, axis=1) + np.tile(
                np.arange(BS), BPC
            )
            ko[:, cum[j] : cum[j] + n] = rows.T.astype(np.int32)
            ln = int(sl[s])
            pos = np.arange(n * CH)
            valid = pos < ln
            rel = np.where(valid, pos - (ln - 1), 0).astype(np.float32)
            msk = np.where(valid, 0.0, NEG).astype(np.float32)
            rm[0, cum[j] * CH : (cum[j] + n) * CH] = rel
            rm[1, cum[j] * CH : (cum[j] + n) * CH] = msk
        in_maps.append(
            {"kc": kc, "vc": vc, "qT": qT, "ko": ko, "rm": rm, "so": so}
        )
    return cnt, assign, in_maps


def kernel(
    query,
    key_cache,
    value_cache,
    num_kv_heads,
    scale,
    block_tables,
    seq_lens,
    block_size,
    max_seq_len,
    alibi_slopes,
):
    global LAST_NC
    from concourse.bass_utils import run_bass_kernel_spmd

    cnt, assign, in_maps = _prep(
        query, key_cache, value_cache, scale, block_tables, seq_lens, alibi_slopes
    )

    if cnt not in _prog_cache:
        _prog_cache[cnt] = _build_program(cnt)
    nc = _prog_cache[cnt]
    LAST_NC = nc

    res = run_bass_kernel_spmd(nc, in_maps, core_ids=list(range(N_CORES)))

    out = np.empty((S, H, D), np.float32)
    for c in range(N_CORES):
        o = res.results[c]["out"]  # [8, 32, 128]
        for j in range(8):
            out[int(assign[j, c])] = o[j]
    return out



# revision 4
# speedup vs baseline: 4.0758x; 4.0758x over previous
"""Paged-attention decode (GQA + ALiBi) Bass kernel for 8 Trainium2 cores.

Problem shape (hardcoded):
  query        [64, 32, 128] f32
  key_cache    [8192, 8, 16, 128] f32
  value_cache  [8192, 8, 16, 128] f32
  block_tables [64, 128] i32
  seq_lens     [64] i32
  out          [64, 32, 128] f32

Strategy: the work is flattened into (seq, chunk-of-128-positions) jobs --
T_all = sum_s ceil(len_s/128) of them -- and dealt contiguously to the 8
cores, C = ceil(T_all/8) jobs each (tail jobs padded with fully-masked
dummies).  One SPMD program value-specialized only on C runs on all cores;
per-job gather indices, ALiBi rel/mask rows and the (scaled, transposed)
query columns stream in as per-core input data.

Per job (chunk of 128 positions = 8 KV blocks x 8 kv heads):
  - K and V arrive via ONE indirect DMA each per group of up to 4 jobs
    (index tile [128, 8*jobs]: partition p=(block,l), column (job,head) ->
    row id block*128 + head*16 + l in the [B*KVH*BS, D] cache view).  One
    descriptor per 512B row; a single Pool/SWDGE instruction covers the
    whole group, amortizing the ~1us per-instruction descriptor-gen cost.
  - per kv head: PE transpose K slab -> K^T; 4 transposes share one
    [128, 512] PSUM tile, copied to SBUF in one shot (DVE for one half,
    ACT for the other, balancing the two engines).
  - bias matmul (rank-2: rel/mask rows x slope/ones) opens the score PSUM
    accumulation, 8 QK matmuls accumulate scoresT [l, 32].
  - ACT exp -> probs [l, 32] (no max-subtraction: logits <= ~10 here,
    masked positions get -1e30 -> exp == 0 exactly).
  - per kv head: PV matmul accumulates outT [d, 32] in PSUM; denominator
    via ones-vector matmul lands in column 32 of the same PSUM tile.
  - one DVE copy moves [128, 33] (outT + den) to an SBUF arena; one DMA
    per group stores the arena to DRAM.
Host epilogue: per-seq segment-sum of the per-job partials, divide by the
summed denominators, transpose [d, h] -> [h, d].  (Partials are linear in
the un-normalized softmax, so chunks of one seq may live on any core.)
"""

import numpy as np

S, H, KVH, GQ, D = 64, 32, 8, 4, 128
BS, NBLOCKS = 16, 8192
N_CORES = 8
CH = 128            # positions per job
BPC = CH // BS      # blocks per job
GC = 4              # max jobs per gather group
NEG = -1.0e30

_prog_cache = {}
LAST_NC = None      # for test harnesses: the last built Bass module


def _build_program(C):
    """Build the SPMD Bass program for C jobs per core."""
    from contextlib import ExitStack

    import concourse.bass as bass
    import concourse.tile as tile
    from concourse import bacc, mybir
    from concourse.masks import make_identity

    f32 = mybir.dt.float32
    i32 = mybir.dt.int32

    # group sizes: full groups of GC plus one remainder group
    groups = [GC] * (C // GC)
    if C % GC:
        groups.append(C % GC)

    nc = bacc.Bacc(
        "TRN2",
        target_bir_lowering=False,
        debug=False,
        enable_asserts=False,
        num_devices=N_CORES,
    )
    kc_d = nc.dram_tensor("kc", [NBLOCKS, KVH, BS, D], f32, kind="ExternalInput")
    vc_d = nc.dram_tensor("vc", [NBLOCKS, KVH, BS, D], f32, kind="ExternalInput")
    qc_d = nc.dram_tensor("qc", [D, C * H], f32, kind="ExternalInput")
    ko_d = nc.dram_tensor("ko", [128, C * BPC], i32, kind="ExternalInput")
    rm_d = nc.dram_tensor("rm", [2, C * CH], f32, kind="ExternalInput")
    so_d = nc.dram_tensor("so", [2, H], f32, kind="ExternalInput")
    po_d = nc.dram_tensor("po", [128, C * 33], f32, kind="ExternalOutput")

    with ExitStack() as ctx:
        tc = ctx.enter_context(tile.TileContext(nc))
        const = ctx.enter_context(tc.tile_pool(name="const", bufs=1))
        kvp = ctx.enter_context(tc.tile_pool(name="kv", bufs=4))
        ktp = ctx.enter_context(tc.tile_pool(name="kt", bufs=3))
        prp = ctx.enter_context(tc.tile_pool(name="pr", bufs=3))
        arp = ctx.enter_context(tc.tile_pool(name="ar", bufs=2))
        psT = ctx.enter_context(tc.tile_pool(name="psT", bufs=2, space="PSUM"))
        psS = ctx.enter_context(tc.tile_pool(name="psS", bufs=2, space="PSUM"))
        psO = ctx.enter_context(tc.tile_pool(name="psO", bufs=2, space="PSUM"))

        ident = const.tile([128, 128], f32)
        make_identity(nc, ident[:])
        ones = const.tile([128, 1], f32)
        nc.gpsimd.memset(ones[:], 1.0)
        qc_s = const.tile([D, C * H], f32)
        nc.sync.dma_start(qc_s[:], qc_d.ap())
        ko_s = const.tile([128, C * BPC], i32)
        nc.sync.dma_start(ko_s[:], ko_d.ap())
        rm_s = const.tile([2, C * CH], f32)
        nc.sync.dma_start(rm_s[:], rm_d.ap())
        so_s = const.tile([2, H], f32)
        nc.sync.dma_start(so_s[:], so_d.ap())

        j0 = 0  # first job of the current group
        for g in groups:
            ksb = kvp.tile([128, GC * KVH * D], f32, tag="k")
            vsb = kvp.tile([128, GC * KVH * D], f32, tag="v")
            for csb, cd in ((ksb, kc_d), (vsb, vc_d)):
                nc.gpsimd.indirect_dma_start(
                    out=csb[:, : g * KVH * D],
                    out_offset=None,
                    in_=cd.ap().rearrange("b h l d -> (b h l) d"),
                    in_offset=bass.IndirectOffsetOnAxis(
                        ap=ko_s[:, j0 * BPC : (j0 + g) * BPC], axis=0
                    ),
                )
            ar = arp.tile([128, GC * 33], f32, tag="ar")
            nc.vector.memset(ar[:], 0.0)
            for tg in range(g):
                j = j0 + tg
                co = tg * KVH * D  # column offset of this job in ksb/vsb
                kt = ktp.tile([128, KVH * D], f32)
                for half in range(2):
                    tp = psT.tile([128, 512], f32, tag="tp")
                    for k in range(4):
                        h = half * 4 + k
                        nc.tensor.transpose(
                            tp[:, k * D : (k + 1) * D],
                            ksb[:, co + h * D : co + (h + 1) * D],
                            ident[:],
                        )
                    dst = kt[:, half * 512 : (half + 1) * 512]
                    if half == 0:
                        nc.vector.tensor_copy(dst, tp[:])
                    else:
                        nc.scalar.activation(
                            dst, tp[:], mybir.ActivationFunctionType.Copy
                        )
                # bias first: one start=True writer for the whole PSUM zero
                # region; QK matmuls then accumulate.
                sc = psS.tile([128, H], f32, tag="sc")
                nc.tensor.matmul(
                    sc[:],
                    lhsT=rm_s[:, j * CH : (j + 1) * CH],
                    rhs=so_s[:],
                    start=True,
                    stop=False,
                )
                for h in range(KVH):
                    nc.tensor.matmul(
                        sc[:, GQ * h : GQ * (h + 1)],
                        lhsT=kt[:, h * D : (h + 1) * D],
                        rhs=qc_s[:, j * H + GQ * h : j * H + GQ * (h + 1)],
                        start=False,
                        stop=h == KVH - 1,
                    )
                pr = prp.tile([128, H], f32)
                nc.scalar.activation(
                    pr[:], sc[:], mybir.ActivationFunctionType.Exp
                )
                po = psO.tile([128, 36], f32, tag="po")
                for h in range(KVH):
                    nc.tensor.matmul(
                        po[:, GQ * h : GQ * (h + 1)],
                        lhsT=vsb[:, co + h * D : co + (h + 1) * D],
                        rhs=pr[:, GQ * h : GQ * (h + 1)],
                        start=h == 0,
                        stop=h == KVH - 1,
                    )
                nc.tensor.matmul(
                    po[0:H, 32:33],
                    lhsT=pr[:],
                    rhs=ones[:],
                    start=True,
                    stop=True,
                )
                nc.vector.tensor_copy(ar[:, tg * 33 : tg * 33 + 32], po[:, 0:32])
                nc.vector.tensor_copy(
                    ar[0:H, tg * 33 + 32 : tg * 33 + 33], po[0:H, 32:33]
                )
            nc.sync.dma_start(
                po_d.ap()[:, j0 * 33 : (j0 + g) * 33], ar[:, : g * 33]
            )
            j0 += g

    nc.compile()
    return nc


def _prep(
    query,
    key_cache,
    value_cache,
    scale,
    block_tables,
    seq_lens,
    alibi_slopes,
):
    q = np.asarray(query, dtype=np.float32)
    kc = np.ascontiguousarray(np.asarray(key_cache, dtype=np.float32))
    vc = np.ascontiguousarray(np.asarray(value_cache, dtype=np.float32))
    bt = np.asarray(block_tables, dtype=np.int32)
    sl = np.asarray(seq_lens, dtype=np.int64)
    slope = np.asarray(alibi_slopes, dtype=np.float32)
    sc_f = float(np.asarray(scale))

    nch = np.maximum(1, -(-sl // CH))  # jobs per seq, >= 1
    jobs = [(s, t) for s in range(S) for t in range(int(nch[s]))]
    C = -(-len(jobs) // N_CORES)

    so = np.stack([slope, np.ones(H, np.float32)]).astype(np.float32)  # [2, 32]
    l_in_p = np.tile(np.arange(BS, dtype=np.int64), BPC)  # [128]: p -> l
    in_maps = []
    core_jobs = []
    for c in range(N_CORES):
        jl = jobs[c * C : (c + 1) * C]
        core_jobs.append(jl)
        qc = np.zeros((D, C * H), np.float32)
        ko = np.zeros((128, C * BPC), np.int32)
        rm = np.zeros((2, C * CH), np.float32)
        rm[1, :] = NEG  # padded jobs: fully masked -> zero contribution
        for j, (s, t) in enumerate(jl):
            qc[:, j * H : (j + 1) * H] = (q[s] * sc_f).T  # [128, 32]
            blk = bt[s, t * BPC : (t + 1) * BPC].astype(np.int64)  # [8]
            # partition p = 16*b + l holds row id blk[b]*128 + h*16 + l of
            # the [NBLOCKS*KVH*BS, D] cache view; one column per (job, h).
            p_rows = np.repeat(blk * KVH * BS, BS) + l_in_p  # [128]
            ko[:, j * BPC : (j + 1) * BPC] = (
                p_rows[:, None] + np.arange(KVH, dtype=np.int64)[None, :] * BS
            ).astype(np.int32)
            ln = int(sl[s])
            pos = t * CH + np.arange(CH)
            valid = pos < ln
            rm[0, j * CH : (j + 1) * CH] = np.where(valid, pos - (ln - 1), 0)
            rm[1, j * CH : (j + 1) * CH] = np.where(valid, 0.0, NEG)
        in_maps.append(
            {"kc": kc, "vc": vc, "qc": qc, "ko": ko, "rm": rm, "so": so}
        )
    return C, core_jobs, in_maps


def kernel(
    query,
    key_cache,
    value_cache,
    num_kv_heads,
    scale,
    block_tables,
    seq_lens,
    block_size,
    max_seq_len,
    alibi_slopes,
):
    global LAST_NC
    from concourse.bass_utils import run_bass_kernel_spmd

    C, core_jobs, in_maps = _prep(
        query, key_cache, value_cache, scale, block_tables, seq_lens, alibi_slopes
    )

    if C not in _prog_cache:
        _prog_cache[C] = _build_program(C)
    nc = _prog_cache[C]
    LAST_NC = nc

    res = run_bass_kernel_spmd(nc, in_maps, core_ids=list(range(N_CORES)))

    acc = np.zeros((S, D, H), np.float64)  # outT partial sums per seq
    den = np.zeros((S, H), np.float64)
    for c in range(N_CORES):
        po = np.asarray(res.results[c]["po"]).reshape(128, C, 33)
        for j, (s, t) in enumerate(core_jobs[c]):
            acc[s] += po[:, j, 0:32]
            den[s] += po[0:H, j, 32]
    out = (acc / den[:, None, :]).transpose(0, 2, 1)  # [S, H, D]
    return np.ascontiguousarray(out.astype(np.float32))


# revision 6
# speedup vs baseline: 4.3435x; 1.0657x over previous
"""Paged-attention decode (GQA + ALiBi) Bass kernel for 8 Trainium2 cores.

Problem shape (hardcoded):
  query        [64, 32, 128] f32
  key_cache    [8192, 8, 16, 128] f32
  value_cache  [8192, 8, 16, 128] f32
  block_tables [64, 128] i32
  seq_lens     [64] i32
  out          [64, 32, 128] f32

Strategy: the work is flattened into (seq, chunk-of-128-positions) jobs --
T_all = sum_s ceil(len_s/128) of them -- and dealt contiguously to the 8
cores, C = ceil(T_all/8) jobs each (tail jobs padded with fully-masked
dummies).  One SPMD program value-specialized only on C runs on all cores;
per-job gather indices, ALiBi rel/mask rows and the (scaled, transposed)
query columns stream in as per-core input data.

Per job (chunk of 128 positions = 8 KV blocks x 8 kv heads):
  - K and V arrive via ONE indirect DMA each per group of up to 4 jobs
    (index tile [128, 8*jobs]: partition p=(block,l), column (job,head) ->
    row id block*128 + head*16 + l in the [B*KVH*BS, D] cache view).  One
    descriptor per 512B row; a single Pool/SWDGE instruction covers the
    whole group, amortizing the ~1us per-instruction descriptor-gen cost.
  - per kv head: PE transpose K slab -> K^T; 4 transposes share one
    [128, 512] PSUM tile, copied to SBUF in one shot (DVE for one half,
    ACT for the other, balancing the two engines).
  - bias matmul (rank-2: rel/mask rows x slope/ones) opens the score PSUM
    accumulation, 8 QK matmuls accumulate scoresT [l, 32].
  - ACT exp -> probs [l, 32] (no max-subtraction: logits <= ~10 here,
    masked positions get -1e30 -> exp == 0 exactly).
  - per kv head: PV matmul accumulates outT [d, 32] in PSUM; denominator
    via ones-vector matmul lands in column 32 of the same PSUM tile.
  - one DVE copy moves [128, 33] (outT + den) to an SBUF arena; one DMA
    per group stores the arena to DRAM.
Host epilogue: per-seq segment-sum of the per-job partials, divide by the
summed denominators, transpose [d, h] -> [h, d].  (Partials are linear in
the un-normalized softmax, so chunks of one seq may live on any core.)
"""

import numpy as np

S, H, KVH, GQ, D = 64, 32, 8, 4, 128
BS, NBLOCKS = 16, 8192
N_CORES = 8
CH = 128            # positions per job
BPC = CH // BS      # blocks per job
GC = 4              # max jobs per gather group
NEG = -1.0e30

_prog_cache = {}
LAST_NC = None      # for test harnesses: the last built Bass module


def _build_program(C):
    """Build the SPMD Bass program for C jobs per core."""
    from contextlib import ExitStack

    import concourse.bass as bass
    import concourse.tile as tile
    from concourse import bacc, mybir
    from concourse.masks import make_identity

    f32 = mybir.dt.float32
    f32r = mybir.dt.float32r
    i32 = mybir.dt.int32

    # group sizes: full groups of GC plus one remainder group
    groups = [GC] * (C // GC)
    if C % GC:
        groups.append(C % GC)

    nc = bacc.Bacc(
        "TRN2",
        target_bir_lowering=False,
        debug=False,
        enable_asserts=False,
        num_devices=N_CORES,
    )
    kc_d = nc.dram_tensor("kc", [NBLOCKS, KVH, BS, D], f32r, kind="ExternalInput")
    vc_d = nc.dram_tensor("vc", [NBLOCKS, KVH, BS, D], f32r, kind="ExternalInput")
    qc_d = nc.dram_tensor("qc", [D, C * H], f32r, kind="ExternalInput")
    ko_d = nc.dram_tensor("ko", [128, C * BPC], i32, kind="ExternalInput")
    rm_d = nc.dram_tensor("rm", [2, C * CH], f32r, kind="ExternalInput")
    so_d = nc.dram_tensor("so", [2, H], f32r, kind="ExternalInput")
    po_d = nc.dram_tensor("po", [128, C * 33], f32, kind="ExternalOutput")

    with ExitStack() as ctx:
        tc = ctx.enter_context(tile.TileContext(nc))
        const = ctx.enter_context(tc.tile_pool(name="const", bufs=1))
        kvp = ctx.enter_context(tc.tile_pool(name="kv", bufs=4))
        ktp = ctx.enter_context(tc.tile_pool(name="kt", bufs=3))
        prp = ctx.enter_context(tc.tile_pool(name="pr", bufs=3))
        arp = ctx.enter_context(tc.tile_pool(name="ar", bufs=2))
        psT = ctx.enter_context(tc.tile_pool(name="psT", bufs=4, space="PSUM"))
        psS = ctx.enter_context(tc.tile_pool(name="psS", bufs=2, space="PSUM"))
        psO = ctx.enter_context(tc.tile_pool(name="psO", bufs=2, space="PSUM"))

        ident = const.tile([128, 128], f32r)
        make_identity(nc, ident[:])
        ones = const.tile([128, 1], f32r)
        nc.gpsimd.memset(ones[:], 1.0)
        qc_s = const.tile([D, C * H], f32r)
        nc.sync.dma_start(qc_s[:], qc_d.ap())
        ko_s = const.tile([128, C * BPC], i32)
        nc.sync.dma_start(ko_s[:], ko_d.ap())
        rm_s = const.tile([2, C * CH], f32r)
        nc.sync.dma_start(rm_s[:], rm_d.ap())
        so_s = const.tile([2, H], f32r)
        nc.sync.dma_start(so_s[:], so_d.ap())

        # Software-pipelined job loop: PV/den/store of job j-1 are emitted
        # after T/QK of job j, so PE never stalls waiting for exp(j-1) and
        # the ACT->PE handoff overlaps the next job's front end.
        deferred = None  # (vsb, co, pr, ar, tg, store_args | None)

        def flush(dfr):
            vsb_, co_, pr_, ar_, tg_, store = dfr
            po = psO.tile([128, 36], f32, tag="po", name="po")
            for h in range(KVH):
                nc.tensor.matmul(
                    po[:, GQ * h : GQ * (h + 1)],
                    lhsT=vsb_[:, co_ + h * D : co_ + (h + 1) * D],
                    rhs=pr_[:, GQ * h : GQ * (h + 1)],
                    start=h == 0,
                    stop=h == KVH - 1,
                )
            nc.tensor.matmul(
                po[0:H, 32:33], lhsT=pr_[:], rhs=ones[:], start=True, stop=True
            )
            nc.vector.tensor_copy(ar_[:, tg_ * 33 : tg_ * 33 + 32], po[:, 0:32])
            nc.vector.tensor_copy(
                ar_[0:H, tg_ * 33 + 32 : tg_ * 33 + 33], po[0:H, 32:33]
            )
            if store is not None:
                dst, src = store
                nc.sync.dma_start(dst, src)

        j0 = 0  # first job of the current group
        for g in groups:
            ksb = kvp.tile([128, GC * KVH * D], f32r, tag="k")
            vsb = kvp.tile([128, GC * KVH * D], f32r, tag="v")
            for csb, cd in ((ksb, kc_d), (vsb, vc_d)):
                nc.gpsimd.indirect_dma_start(
                    out=csb[:, : g * KVH * D],
                    out_offset=None,
                    in_=cd.ap().rearrange("b h l d -> (b h l) d"),
                    in_offset=bass.IndirectOffsetOnAxis(
                        ap=ko_s[:, j0 * BPC : (j0 + g) * BPC], axis=0
                    ),
                )
            ar = arp.tile([128, GC * 33], f32, tag="ar")
            nc.vector.memset(ar[:], 0.0)
            for tg in range(g):
                j = j0 + tg
                co = tg * KVH * D  # column offset of this job in ksb/vsb
                kt = ktp.tile([128, KVH * D], f32r)
                for half in range(2):
                    tp = psT.tile([128, 512], f32r, tag="tp")
                    for k in range(4):
                        h = half * 4 + k
                        nc.tensor.transpose(
                            tp[:, k * D : (k + 1) * D],
                            ksb[:, co + h * D : co + (h + 1) * D],
                            ident[:],
                        )
                    dst = kt[:, half * 512 : (half + 1) * 512]
                    if half == 0:
                        nc.vector.tensor_copy(dst, tp[:])
                    else:
                        nc.scalar.activation(
                            dst, tp[:], mybir.ActivationFunctionType.Copy
                        )
                # bias first: one start=True writer for the whole PSUM zero
                # region; QK matmuls then accumulate.
                sc = psS.tile([128, H], f32, tag="sc")
                nc.tensor.matmul(
                    sc[:],
                    lhsT=rm_s[:, j * CH : (j + 1) * CH],
                    rhs=so_s[:],
                    start=True,
                    stop=False,
                )
                for h in range(KVH):
                    nc.tensor.matmul(
                        sc[:, GQ * h : GQ * (h + 1)],
                        lhsT=kt[:, h * D : (h + 1) * D],
                        rhs=qc_s[:, j * H + GQ * h : j * H + GQ * (h + 1)],
                        start=False,
                        stop=h == KVH - 1,
                    )
                pr = prp.tile([128, H], f32r)
                nc.scalar.activation(
                    pr[:], sc[:], mybir.ActivationFunctionType.Exp
                )
                if deferred is not None:
                    flush(deferred)
                store = None
                if tg == g - 1:
                    store = (
                        po_d.ap()[:, j0 * 33 : (j0 + g) * 33],
                        ar[:, : g * 33],
                    )
                deferred = (vsb, co, pr, ar, tg, store)
            j0 += g
        flush(deferred)

    nc.compile()
    return nc


def _prep(
    query,
    key_cache,
    value_cache,
    scale,
    block_tables,
    seq_lens,
    alibi_slopes,
):
    q = np.asarray(query, dtype=np.float32)
    kc = np.ascontiguousarray(np.asarray(key_cache, dtype=np.float32))
    vc = np.ascontiguousarray(np.asarray(value_cache, dtype=np.float32))
    bt = np.asarray(block_tables, dtype=np.int32)
    sl = np.asarray(seq_lens, dtype=np.int64)
    slope = np.asarray(alibi_slopes, dtype=np.float32)
    sc_f = float(np.asarray(scale))

    nch = np.maximum(1, -(-sl // CH))  # jobs per seq, >= 1
    jobs = [(s, t) for s in range(S) for t in range(int(nch[s]))]
    C = -(-len(jobs) // N_CORES)

    so = np.stack([slope, np.ones(H, np.float32)]).astype(np.float32)  # [2, 32]
    l_in_p = np.tile(np.arange(BS, dtype=np.int64), BPC)  # [128]: p -> l
    in_maps = []
    core_jobs = []
    for c in range(N_CORES):
        jl = jobs[c * C : (c + 1) * C]
        core_jobs.append(jl)
        qc = np.zeros((D, C * H), np.float32)
        ko = np.zeros((128, C * BPC), np.int32)
        rm = np.zeros((2, C * CH), np.float32)
        rm[1, :] = NEG  # padded jobs: fully masked -> zero contribution
        for j, (s, t) in enumerate(jl):
            qc[:, j * H : (j + 1) * H] = (q[s] * sc_f).T  # [128, 32]
            blk = bt[s, t * BPC : (t + 1) * BPC].astype(np.int64)  # [8]
            # partition p = 16*b + l holds row id blk[b]*128 + h*16 + l of
            # the [NBLOCKS*KVH*BS, D] cache view; one column per (job, h).
            p_rows = np.repeat(blk * KVH * BS, BS) + l_in_p  # [128]
            ko[:, j * BPC : (j + 1) * BPC] = (
                p_rows[:, None] + np.arange(KVH, dtype=np.int64)[None, :] * BS
            ).astype(np.int32)
            ln = int(sl[s])
            pos = t * CH + np.arange(CH)
            valid = pos < ln
            rm[0, j * CH : (j + 1) * CH] = np.where(valid, pos - (ln - 1), 0)
            rm[1, j * CH : (j + 1) * CH] = np.where(valid, 0.0, NEG)
        in_maps.append(
            {"kc": kc, "vc": vc, "qc": qc, "ko": ko, "rm": rm, "so": so}
        )
    return C, core_jobs, in_maps


def kernel(
    query,
    key_cache,
    value_cache,
    num_kv_heads,
    scale,
    block_tables,
    seq_lens,
    block_size,
    max_seq_len,
    alibi_slopes,
):
    global LAST_NC
    from concourse.bass_utils import run_bass_kernel_spmd

    C, core_jobs, in_maps = _prep(
        query, key_cache, value_cache, scale, block_tables, seq_lens, alibi_slopes
    )

    if C not in _prog_cache:
        _prog_cache[C] = _build_program(C)
    nc = _prog_cache[C]
    LAST_NC = nc

    res = run_bass_kernel_spmd(nc, in_maps, core_ids=list(range(N_CORES)))

    acc = np.zeros((S, D, H), np.float64)  # outT partial sums per seq
    den = np.zeros((S, H), np.float64)
    for c in range(N_CORES):
        po = np.asarray(res.results[c]["po"]).reshape(128, C, 33)
        for j, (s, t) in enumerate(core_jobs[c]):
            acc[s] += po[:, j, 0:32]
            den[s] += po[0:H, j, 32]
    out = (acc / den[:, None, :]).transpose(0, 2, 1)  # [S, H, D]
    return np.ascontiguousarray(out.astype(np.float32))
